# revision 19
# baseline (speedup 1.0000x reference)
"""Masked self-attention Trainium2 Bass kernel.

Reference computation (per batch b):
    q = x @ Wq + bq ; k = x @ Wk + bk ; v = x @ Wv + bv      # [S, A]
    scores = (q @ k.T) / sqrt(S)  with causal mask            # [S, S]
    out = softmax(scores, axis=-1) @ v                        # [S, A]

Sharding: data-parallel over batch across 8 NeuronCores (B=32 -> 4 per core),
weights replicated. No collectives.

Per-core design (matmuls in fp32r = full-rate PE fp32; fp32r moving dim must
be even; fp32r operands must be produced by a rounding compute op, so DMA-fed
operands are staged fp32 and rounded by DVE/ACT):
  Stage A: DMA x[b] [S,E]; PE-transpose via identity -> PSUM; DVE copy to
           xT [E,S] fp32r in SBUF.
  Stage B: qT = Wq.T @ xT -> [A,S]; bias + 1/sqrt(S) folded into the ACT
           PSUM->SBUF copy (per-partition bias AP). kT likewise. v = xT.T @ Wv
           -> [S,A] (no bias: since softmax rows sum to 1, bv is added to the
           final output instead). v tiles carry 2 extra ones-columns.
  Stage C: scoresT[k,q] = kT.T @ qT per k-tile, causal-trimmed even chunks;
           additive -1e9 mask on the diagonal block in PSUM (DVE); exp on ACT
           (PSUM->SBUF, rounds to fp32r). No max-subtraction: |scores| <~ 3.
  Stage D: out_psum = sum_t expT[t].T @ v_aug[t] in two column chunks; the
           ones-columns yield the softmax row-sums; DVE reciprocal; ACT copy
           scales rows by 1/sum; DVE adds broadcast bv; DMA out.
"""

import numpy as np
from contextlib import ExitStack

import concourse.bass as bass
import concourse.mybir as mybir
import concourse.tile as tile
from concourse import bacc
from concourse.bass_utils import run_bass_kernel_spmd
from concourse.masks import make_identity

P = 128
F32 = mybir.dt.float32
F32R = mybir.dt.float32r
AF = mybir.ActivationFunctionType

N_CORES = 8
B, S, E, A = 32, 1000, 1024, 512
MASK_NEG = -1.0e9


def _even_chunks(start, total, maxc):
    """Split [start, start+total) into ceil(total/maxc) near-even chunks,
    each of even size (required by fp32r matmul moving dim)."""
    assert total % 2 == 0
    n = max(1, -(-total // maxc))
    bounds = [start + ((i * total) // n) // 2 * 2 for i in range(n)]
    bounds.append(start + total)
    return [(bounds[i], bounds[i + 1] - bounds[i]) for i in range(n)]


def build(b_pc, s, e, a, reps=1):
    assert e % P == 0 and a % P == 0
    n_s = -(-s // P)
    n_e = e // P
    n_a = a // P
    inv_den = float(s) ** -0.5
    s_tiles = [(t * P, min(P, s - t * P)) for t in range(n_s)]
    h = a // 2  # PV column split: [0,h) and [h, a+2)

    nc = bacc.Bacc("TRN2")
    x = nc.dram_tensor("x", [b_pc, s, e], F32R, kind="ExternalInput").ap()
    w_dram = {
        "q": nc.dram_tensor("Wq", [e, a], F32, kind="ExternalInput").ap(),
        "k": nc.dram_tensor("Wk", [e, a], F32, kind="ExternalInput").ap(),
        "v": nc.dram_tensor("Wv", [e, a], F32, kind="ExternalInput").ap(),
    }
    b_dram = {
        "q": nc.dram_tensor("bq", [a], F32, kind="ExternalInput").ap(),
        "k": nc.dram_tensor("bk", [a], F32, kind="ExternalInput").ap(),
        "v": nc.dram_tensor("bv", [a], F32, kind="ExternalInput").ap(),
    }
    out = nc.dram_tensor("out", [b_pc, s, a], F32, kind="ExternalOutput").ap()

    with tile.TileContext(nc) as tc, ExitStack() as ctx:
        pool = ctx.enter_context(tc.tile_pool(name="sb", bufs=1))
        pp_tp = ctx.enter_context(tc.tile_pool(name="pp_tp", bufs=2, space="PSUM"))
        pp_proj = ctx.enter_context(tc.tile_pool(name="pp_proj", bufs=2, space="PSUM"))
        pp_score = ctx.enter_context(tc.tile_pool(name="pp_sc", bufs=2, space="PSUM"))
        pp_o1 = ctx.enter_context(tc.tile_pool(name="pp_o1", bufs=1, space="PSUM"))
        pp_o2 = ctx.enter_context(tc.tile_pool(name="pp_o2", bufs=1, space="PSUM"))

        # ---------------- constants ----------------
        ident_st = pool.tile([P, P], F32)
        make_identity(nc, ident_st)
        ident = pool.tile([P, P], F32R)
        nc.scalar.copy(ident[:], ident_st[:])

        # additive causal mask for the diagonal block:
        # keep 0 where col q >= row k (i.e. (y - x) >= 0), else fill -1e9
        amask = pool.tile([P, P], F32)
        nc.gpsimd.memset(amask, 0.0)
        nc.gpsimd.affine_select(
            out=amask, in_=amask,
            compare_op=mybir.AluOpType.is_ge,
            fill=MASK_NEG, base=0,
            pattern=[[1, P]], channel_multiplier=-1,
        )

        ones_stage = pool.tile([P, 2], F32)
        nc.gpsimd.memset(ones_stage, 1.0)

        # ---------------- weights / biases ----------------
        w_sb = {}
        for nm in ("q", "k", "v"):
            tiles = []
            for u in range(n_e):
                w_stage = pool.tile([P, a], F32, tag="w_stage", bufs=2)
                nc.gpsimd.dma_start(w_stage[:], w_dram[nm][u * P:(u + 1) * P, :])
                w_r = pool.tile([P, a], F32R, tag=f"w_{nm}", bufs=n_e)
                nc.vector.tensor_copy(w_r[:], w_stage[:])
                tiles.append(w_r)
            w_sb[nm] = tiles

        bias_sb = {}
        for nm in ("q", "k"):
            b_st = pool.tile([P, n_a], F32, tag=f"b_{nm}", bufs=1)
            nc.gpsimd.dma_start(
                b_st[:], b_dram[nm].rearrange("(m p) -> p m", p=P)
            )
            bias_sb[nm] = b_st
        # pre-scale bq by 1/sqrt(S) (scores scaling folded into q)
        bqs = pool.tile([P, n_a], F32)
        nc.scalar.mul(bqs[:], bias_sb["q"][:], inv_den)
        bias_sb["q"] = bqs

        bv_stage = pool.tile([1, a], F32)
        nc.gpsimd.dma_start(bv_stage[:], b_dram["v"][:])
        bv_bc = pool.tile([P, a], F32)
        nc.gpsimd.partition_broadcast(bv_bc[:], bv_stage[:])

        # ---------------- per-batch pipeline ----------------
        # reps>1 wraps the whole pipeline in an on-device loop re-running the
        # same work — used only to measure device exec time (amortizes the
        # per-dispatch RPC overhead, which otherwise hides the kernel).
        rep_ctx = tc.For_i(0, reps, 1) if reps > 1 else None
        if rep_ctx is not None:
            ctx.enter_context(rep_ctx)
        for b in range(b_pc):
            # ---- stage A: load x, transpose to xT [E, S] ----
            xT = [pool.tile([P, s], F32R, tag="xT", bufs=n_e + 2,
                            name=f"xT{b}_{u}")
                  for u in range(n_e)]
            for (s0, sl) in s_tiles:
                x_sb = pool.tile([P, e], F32R, tag="x", bufs=3)
                # split the load across DMA queues for parallelism; finer
                # split for the first batch, whose loads pace the pipeline fill
                nsp = 4 if b == 0 else 2
                w_sp = e // nsp
                for qi in range(nsp):
                    nc.sync.dma_start(
                        x_sb[:sl, qi * w_sp:(qi + 1) * w_sp],
                        x[b, s0:s0 + sl, qi * w_sp:(qi + 1) * w_sp],
                    )
                for u in range(n_e):
                    tp = pp_tp.tile([P, P], F32R, tag="tp")
                    nc.tensor.transpose(
                        tp[:, :sl], x_sb[:sl, u * P:(u + 1) * P], ident[:sl, :sl]
                    )
                    nc.vector.tensor_copy(xT[u][:, s0:s0 + sl], tp[:, :sl])

            # ---- stage B: projections ----
            # qT/kT [A, S] (a on partitions)
            qkT = {}
            for nm in ("q", "k"):
                scale = inv_den if nm == "q" else 1.0
                tiles = []
                for m in range(n_a):
                    dest = pool.tile([P, s], F32R, tag=f"{nm}T", bufs=n_a,
                                     name=f"{nm}T{b}_{m}")
                    tiles.append(dest)
                    for (c0, cl) in _even_chunks(0, s, 512):
                        mm = pp_proj.tile([P, 512], F32, tag="proj")
                        for u in range(n_e):
                            nc.tensor.matmul(
                                mm[:, :cl],
                                w_sb[nm][u][:, m * P:(m + 1) * P],
                                xT[u][:, c0:c0 + cl],
                                start=(u == 0), stop=(u == n_e - 1),
                            )
                        nc.scalar.activation(
                            dest[:, c0:c0 + cl], mm[:, :cl], AF.Identity,
                            bias=bias_sb[nm][:, m:m + 1], scale=scale,
                        )
                qkT[nm] = tiles

            # v [S, A+2] natural layout; last two columns are ones (for the
            # softmax row-sums via the PV matmul)
            v_tiles = []
            for (s0, sl) in s_tiles:
                vm = pp_proj.tile([P, 512], F32, tag="proj")
                for u in range(n_e):
                    nc.tensor.matmul(
                        vm[:sl, :a], xT[u][:, s0:s0 + sl], w_sb["v"][u][:],
                        start=(u == 0), stop=(u == n_e - 1),
                    )
                v_t = pool.tile([P, a + 2], F32R, tag="v", bufs=n_s)
                nc.vector.tensor_copy(v_t[:sl, :a], vm[:sl, :a])
                nc.scalar.copy(v_t[:sl, a:a + 2], ones_stage[:sl, :])
                v_tiles.append(v_t)

            # ---- stages C+D interleaved per tile: scoresT/exp for k-tile
            # t, then PV/out for q-tile t (its expT deps are all ready) ----
            expT = []
            for t, (k0, kl) in enumerate(s_tiles):
                et = pool.tile([P, s - k0], F32R, tag=f"expT{t}", bufs=1,
                               name=f"et{b}_{t}")
                expT.append(et)
                for pi, (c0, cl) in enumerate(_even_chunks(k0, s - k0, 512)):
                    sc = pp_score.tile([P, 512], F32, tag="score")
                    for m in range(n_a):
                        nc.tensor.matmul(
                            sc[:kl, :cl],
                            qkT["k"][m][:, k0:k0 + kl],
                            qkT["q"][m][:, c0:c0 + cl],
                            start=(m == 0), stop=(m == n_a - 1),
                        )
                    if pi == 0:
                        # diagonal block: additive causal mask in PSUM
                        nc.vector.tensor_add(
                            sc[:kl, :kl], sc[:kl, :kl], amask[:kl, :kl]
                        )
                    nc.scalar.activation(
                        et[:kl, c0 - k0:c0 - k0 + cl], sc[:kl, :cl], AF.Exp,
                    )

                i, (q0, il) = t, s_tiles[t]
                op1 = pp_o1.tile([P, h], F32, tag="op1")
                op2 = pp_o2.tile([P, a - h + 2], F32, tag="op2")
                for t in range(i + 1):
                    k0t, klt = s_tiles[t]
                    lhs = expT[t][:klt, q0 - k0t:q0 - k0t + il]
                    nc.tensor.matmul(
                        op1[:il, :], lhs, v_tiles[t][:klt, 0:h],
                        start=(t == 0), stop=(t == i),
                    )
                    nc.tensor.matmul(
                        op2[:il, :], lhs, v_tiles[t][:klt, h:a + 2],
                        start=(t == 0), stop=(t == i),
                    )
                rec = pool.tile([P, 1], F32, tag="rec", bufs=2)
                nc.vector.reciprocal(rec[:il, :], op2[:il, a - h:a - h + 1])
                o_sb = pool.tile([P, a], F32, tag="o_sb", bufs=3)
                nc.scalar.activation(
                    o_sb[:il, 0:h], op1[:il, :], AF.Identity,
                    bias=0.0, scale=rec[:il, 0:1],
                )
                nc.scalar.activation(
                    o_sb[:il, h:a], op2[:il, 0:a - h], AF.Identity,
                    bias=0.0, scale=rec[:il, 0:1],
                )
                nc.gpsimd.tensor_add(o_sb[:il, :], o_sb[:il, :], bv_bc[:il, :])
                nc.sync.dma_start(out[b, q0:q0 + il, :], o_sb[:il, :])

    nc.compile()
    return nc


_BUILT = {}


def _get_nc(b_pc, s, e, a):
    key = (b_pc, s, e, a)
    if key not in _BUILT:
        _BUILT[key] = build(b_pc, s, e, a)
    return _BUILT[key]


def run_sharded(inputs, b_pc, s, e, a, **run_kwargs):
    """Run the SPMD kernel over N_CORES cores, sharding batch dim of x."""
    x = np.ascontiguousarray(inputs["x"], dtype=np.float32)
    b_total = x.shape[0]
    assert b_total == b_pc * N_CORES
    shared = {
        "Wq": np.ascontiguousarray(inputs["Wq"], dtype=np.float32),
        "Wk": np.ascontiguousarray(inputs["Wk"], dtype=np.float32),
        "Wv": np.ascontiguousarray(inputs["Wv"], dtype=np.float32),
        "bq": np.ascontiguousarray(inputs["bq"], dtype=np.float32),
        "bk": np.ascontiguousarray(inputs["bk"], dtype=np.float32),
        "bv": np.ascontiguousarray(inputs["bv"], dtype=np.float32),
    }
    in_maps = [
        {"x": x[c * b_pc:(c + 1) * b_pc], **shared} for c in range(N_CORES)
    ]
    nc = _get_nc(b_pc, s, e, a)
    res = run_bass_kernel_spmd(nc, in_maps, core_ids=list(range(N_CORES)),
                               **run_kwargs)
    full = np.concatenate([res.results[c]["out"] for c in range(N_CORES)], axis=0)
    return full, res


def kernel(**inputs) -> np.ndarray:
    out, _ = run_sharded(inputs, B // N_CORES, S, E, A)
    return out
